# revision 16
# baseline (speedup 1.0000x reference)
"""Trainium2 Bass kernel for nn_CDALayer (squeeze-excitation-style gated MLP).

Computes: y0 = mean(x, axis=(2,3)); tiny cross-linked MLP -> sigmoid gate;
out = x * gate[:, :, None, None].

Strategy: data-parallel over batch (32 -> 4 per core x 8 cores). Each core
keeps its full 16 MiB x-shard resident in SBUF so x is read from HBM exactly
once (16 MiB in + 16 MiB out per core ~ the memory roofline). The MLP's
linear cross-links are folded host-side:
  y1   = relu(y0 @ (w0_1 + w01).T)
  y2   = relu(y1 @ (w0_2 + w12).T + y0 @ w02.T)
  gate = sigmoid(y2 @ (w0_3 + w23).T + y1 @ w13.T + y0 @ w03.T)
and the 1/(H*W) mean scale is folded into every weight that consumes y0,
so the device only needs row sums of x.

The kernel is pipelined per batch image: loads are issued batch-major and
chunked so row-sum reduces track the data as it arrives, each image's tiny
MLP runs as soon as its two channel-half sums are ready, and gated
multiplies + stores for image b overlap the loads of images b+1.. so the
DMA engines (the roofline resource) stay continuously busy. Reduce and
multiply work is split between the Vector and Scalar engines.
"""

import sys

if "/opt/trn_rl_repo" not in sys.path:
    sys.path.insert(0, "/opt/trn_rl_repo")

import numpy as np

import concourse.bacc as bacc
import concourse.tile as tile
from concourse import mybir
from concourse.bass_utils import run_bass_kernel_spmd

N_CORES = 8
B, C, H, W = 32, 256, 64, 64
BPC = B // N_CORES  # 4 images per core
HW = H * W  # 4096
CR = 16
NH = C // 128  # 2 channel halves (partition tiles)
LOAD_CHUNKS = 2
CHW = HW // LOAD_CHUNKS
F32 = mybir.dt.float32
AF = mybir.ActivationFunctionType

_CACHED = {}


def _build_bass(unroll=1):
    nc = bacc.Bacc("TRN2", target_bir_lowering=False, num_devices=N_CORES)

    x_d = nc.declare_dram_parameter("x", [BPC, NH, 128, HW], F32, isOutput=False)
    w1_d = nc.declare_dram_parameter("wu1t", [128, NH, CR], F32, isOutput=False)
    w2a_d = nc.declare_dram_parameter("wu2at", [CR, CR], F32, isOutput=False)
    w2b_d = nc.declare_dram_parameter("wu2bt", [128, NH, CR], F32, isOutput=False)
    w3a_d = nc.declare_dram_parameter("wu3at", [CR, C], F32, isOutput=False)
    w3b_d = nc.declare_dram_parameter("wu3bt", [CR, C], F32, isOutput=False)
    w3c_d = nc.declare_dram_parameter("wu3ct", [128, NH, C], F32, isOutput=False)
    out_d = nc.declare_dram_parameter("out", [BPC, NH, 128, HW], F32, isOutput=True)

    with tile.TileContext(nc) as tc:
        with (
            tc.tile_pool(name="xpool", bufs=BPC * NH) as xpool,
            tc.tile_pool(name="singles", bufs=1) as singles,
            tc.tile_pool(name="small", bufs=2) as small,
            tc.tile_pool(name="psum", bufs=2, space="PSUM") as psum,
        ):
            # Warm the ACT function table with the set that covers
            # Copy/Relu/Sigmoid so no mid-pipeline table load happens.
            warm = singles.tile([1, 1], F32, tag="warm")
            nc.gpsimd.memset(warm, 0.0)
            nc.scalar.activation(out=warm, in_=warm, func=AF.Sigmoid)

            # Weight loads ride the gpsimd/SWDGE path so the sync/HWDGE path
            # is free to start streaming x immediately.
            w1_sb = singles.tile([128, NH, CR], F32, tag="w1")
            nc.gpsimd.dma_start(out=w1_sb, in_=w1_d[:])
            w2a_sb = singles.tile([CR, CR], F32, tag="w2a")
            nc.gpsimd.dma_start(out=w2a_sb, in_=w2a_d[:])
            w2b_sb = singles.tile([128, NH, CR], F32, tag="w2b")
            nc.gpsimd.dma_start(out=w2b_sb, in_=w2b_d[:])
            w3a_sb = singles.tile([CR, C], F32, tag="w3a")
            nc.gpsimd.dma_start(out=w3a_sb, in_=w3a_d[:])
            w3b_sb = singles.tile([CR, C], F32, tag="w3b")
            nc.gpsimd.dma_start(out=w3b_sb, in_=w3b_d[:])
            w3c_sb = singles.tile([128, NH, C], F32, tag="w3c")
            nc.gpsimd.dma_start(out=w3c_sb, in_=w3c_d[:])

            # unroll>1 repeats the whole body (bench-only; same output).
            for _it in range(unroll):
                _body(nc, xpool, small, psum, x_d, out_d,
                      w1_sb, w2a_sb, w2b_sb, w3a_sb, w3b_sb, w3c_sb)

    nc.compile()
    return nc


def _body(nc, xpool, small, psum, x_d, out_d,
          w1_sb, w2a_sb, w2b_sb, w3a_sb, w3b_sb, w3c_sb):
    # y0T[c_in_half, h, b] holds per-(batch, channel) SUMS over H*W
    y0_sb = small.tile([128, NH, BPC], F32, tag="y0", name="y0_sb")

    # Batch-major chunked loads; per-chunk partial row sums track the
    # data as it arrives (so the last image's sums are nearly done
    # when its final chunk lands, whatever order the DMA queues
    # complete in). Reduce work splits between DVE (reduce_sum, h=0)
    # and ACT (in-place Copy with accum_out, h=1).
    xt = [[None] * NH for _ in range(BPC)]
    for b in range(BPC):
        pt = small.tile([128, NH, LOAD_CHUNKS], F32, tag="part",
                        name=f"part_{b}")
        for h in range(NH):
            t = xpool.tile([128, HW], F32, tag="xbig", name=f"x_{b}_{h}")
            for c in range(LOAD_CHUNKS):
                cs = slice(c * CHW, (c + 1) * CHW)
                nc.sync.dma_start(out=t[:, cs], in_=x_d[b, h, :, cs])
                if h == 0:
                    nc.vector.reduce_sum(
                        out=pt[:, h, c : c + 1], in_=t[:, cs],
                        axis=mybir.AxisListType.X,
                    )
                else:
                    nc.scalar.activation(
                        out=t[:, cs], in_=t[:, cs], func=AF.Copy,
                        accum_out=pt[:, h, c : c + 1],
                    )
            xt[b][h] = t
        # Combine the chunk partials into y0 (same engine split).
        nc.vector.reduce_sum(
            out=y0_sb[:, 0, b : b + 1], in_=pt[:, 0, :],
            axis=mybir.AxisListType.X,
        )
        nc.scalar.activation(
            out=pt[:, 1, :], in_=pt[:, 1, :], func=AF.Copy,
            accum_out=y0_sb[:, 1, b : b + 1],
        )

    # Per-batch MLP on PE + gated multiply + store, pipelined so
    # image b's stores overlap images b+1.. loads.
    for b in range(BPC):
        bs = slice(b, b + 1)
        z1_ps = psum.tile([CR, 1], F32, tag="z1", name=f"z1_{b}")
        nc.tensor.matmul(z1_ps, w1_sb[:, 0, :], y0_sb[:, 0, bs], start=True, stop=False)
        nc.tensor.matmul(z1_ps, w1_sb[:, 1, :], y0_sb[:, 1, bs], start=False, stop=True)
        y1_sb = small.tile([CR, 1], F32, tag="y1", name=f"y1_{b}")
        nc.scalar.activation(out=y1_sb, in_=z1_ps, func=AF.Relu)

        z2_ps = psum.tile([CR, 1], F32, tag="z2", name=f"z2_{b}")
        nc.tensor.matmul(z2_ps, w2a_sb, y1_sb, start=True, stop=False)
        nc.tensor.matmul(z2_ps, w2b_sb[:, 0, :], y0_sb[:, 0, bs], start=False, stop=False)
        nc.tensor.matmul(z2_ps, w2b_sb[:, 1, :], y0_sb[:, 1, bs], start=False, stop=True)
        y2_sb = small.tile([CR, 1], F32, tag="y2", name=f"y2_{b}")
        nc.scalar.activation(out=y2_sb, in_=z2_ps, func=AF.Relu)

        g_sb = small.tile([128, NH], F32, tag="g", name=f"g_{b}")
        for mh in range(NH):
            ms = slice(mh * 128, (mh + 1) * 128)
            z3_ps = psum.tile([128, 1], F32, tag=f"z3_{mh}", name=f"z3_{b}_{mh}")
            nc.tensor.matmul(z3_ps, w3a_sb[:, ms], y2_sb, start=True, stop=False)
            nc.tensor.matmul(z3_ps, w3b_sb[:, ms], y1_sb, start=False, stop=False)
            nc.tensor.matmul(z3_ps, w3c_sb[:, 0, ms], y0_sb[:, 0, bs], start=False, stop=False)
            nc.tensor.matmul(z3_ps, w3c_sb[:, 1, ms], y0_sb[:, 1, bs], start=False, stop=True)
            nc.scalar.activation(out=g_sb[:, mh : mh + 1], in_=z3_ps, func=AF.Sigmoid)

        # Gated multiply + store, chunked so the first store bytes
        # trail the gate by one chunk's multiply, not a whole tile's.
        for h in range(NH):
            t = xt[b][h]
            gv = g_sb[:, h : h + 1]
            for c in range(LOAD_CHUNKS):
                cs = slice(c * CHW, (c + 1) * CHW)
                if h == 1:
                    nc.vector.tensor_scalar_mul(
                        out=t[:, cs], in0=t[:, cs], scalar1=gv)
                else:
                    nc.scalar.mul(out=t[:, cs], in_=t[:, cs], mul=gv)
                nc.sync.dma_start(out=out_d[b, h, :, cs], in_=t[:, cs])


def _prep_weights(w0_1, w0_2, w0_3, w01, w02, w03, w12, w13, w23):
    inv = np.float32(1.0 / HW)
    u1 = (w0_1 + w01) * inv  # [CR, C], consumes y0 sums
    u2a = w0_2 + w12  # [CR, CR]
    u2b = w02 * inv  # [CR, C]
    u3a = w0_3 + w23  # [C, CR]
    u3b = w13  # [C, CR]
    u3c = w03 * inv  # [C, C]

    def t_khalf(u):  # [out, C] -> lhsT layout [128, NH, out]
        return np.ascontiguousarray(
            u.T.reshape(NH, 128, u.shape[0]).transpose(1, 0, 2)
        ).astype(np.float32)

    return {
        "wu1t": t_khalf(u1),
        "wu2at": np.ascontiguousarray(u2a.T).astype(np.float32),
        "wu2bt": t_khalf(u2b),
        "wu3at": np.ascontiguousarray(u3a.T).astype(np.float32),
        "wu3bt": np.ascontiguousarray(u3b.T).astype(np.float32),
        "wu3ct": t_khalf(u3c),
    }


def kernel(run_opts=None, **inputs):
    x = np.asarray(inputs["x"], dtype=np.float32)
    assert x.shape == (B, C, H, W), x.shape

    weights = _prep_weights(
        *(np.asarray(inputs[k], dtype=np.float32)
          for k in ("w0_1", "w0_2", "w0_3", "w01", "w02", "w03", "w12", "w13", "w23"))
    )

    if "nc" not in _CACHED:
        _CACHED["nc"] = _build_bass()
    nc = _CACHED["nc"]

    xv = x.reshape(B, NH, 128, HW)
    in_maps = [
        {"x": xv[c * BPC : (c + 1) * BPC], **weights} for c in range(N_CORES)
    ]
    res = run_bass_kernel_spmd(nc, in_maps, core_ids=list(range(N_CORES)),
                               **(run_opts or {}))
    out = np.concatenate([r["out"].reshape(BPC, C, H, W) for r in res.results], axis=0)
    if run_opts:
        _CACHED["last_result"] = res
    return out
